# revision 1
# baseline (speedup 1.0000x reference)
"""XNOR-Net style binarized 3x3 conv (BinConv2d) on 8 Trainium2 NeuronCores.

Math: out = conv(sign(x)*mean|x|, sign(w)*mean|w|) + b
         = (mean|x| * mean|w|) * conv(sign(x), sign(w)) + b

The conv operands are pure {-1, 0, +1}, exactly representable in fp8e4m3, and
all partial sums are small integers held exactly in fp32 PSUM, so the heavy
conv runs on the tensor engine in fp8 with DoubleRow (2x) perf mode.

Layout (per core, data-parallel over batch: 4 images/core):
  - input ships as bf16 (sign() is invariant under bf16 rounding), sign is
    computed on ScalarE into a padded fp8 image: 58 rows x 58 cols (pad=1
    ring of zeros), flattened per channel partition.
  - conv = 9 shifted matmuls over the flat image with C=128 as the
    partition/contraction dim, accumulated in PSUM; spatial tiles are 8
    padded rows (464 positions) so tile boundaries align with output rows.
  - the 9 kernel taps run as 4 DoubleRow pair-matmuls + 1 plain matmul.
  - PSUM eviction fuses scale (mean|x|*mean|w|) + bias and drops the pad
    columns (VectorE tensor_scalar / ScalarE Identity), so the output
    staging buffer is dense and the output DMA is fully contiguous.

Measured ~70-80us/core end-to-end on HW (PE-bound at the fp8 DoubleRow
rate; input/output DMA, sign, and evictions fully overlapped).
"""

import numpy as np
import ml_dtypes

# Problem constants (hardcoded per contest rules)
N, C, H, W = 32, 128, 56, 56
K, KS = 256, 3
NCORES = 8
NPC = N // NCORES          # images per core
RS = H + 2                 # padded row stride = 58
PLANE = RS * RS            # 3364
IMG_LEN = 3712             # >= 59 + 6*512 + 511 + 59 + 1 = 3702, rounded up
ALPHA = 16                 # image plane base offset inside the SBUF buffer
TS = 8 * RS                # spatial tile = 8 padded rows = 464 (psum free dim)
NTILE = 7                  # 7 row-aligned tiles cover padded rows 1..56
ROWS_PT = 8                # output rows per tile

# tap order: pairs (0,1),(2,3),(4,5),(6,7) are DoubleRow pairs; 8 is single
ORD = [(-1, -1), (-1, 0), (-1, 1), (0, -1), (0, 0), (0, 1), (1, -1), (1, 0), (1, 1)]
OFF = [dy * RS + dx for (dy, dx) in ORD]


def build_program(scale: float, loop_n: int | None = None, psum_bufs: int = 6,
                  os_bufs: int = 6, sign_split: int = 2, evict: str = "dve",
                  out_dma: bool = True, in_path: bool = True,
                  img_bufs: int = 2, raw_bufs: int = 2):
    """Build the per-core program. loop_n: if set, wrap the whole body in a
    hardware For loop repeating it loop_n times (timing-harness variant)."""
    import contextlib
    from concourse import bass, bacc, tile, mybir

    FP8 = mybir.dt.float8e4
    F32 = mybir.dt.float32
    DR = mybir.MatmulPerfMode.DoubleRow
    ACT_ID = mybir.ActivationFunctionType.Identity
    ACT_SIGN = mybir.ActivationFunctionType.Sign

    BF16 = mybir.dt.bfloat16
    nc = bacc.Bacc("TRN2", target_bir_lowering=False, debug=False)
    x_d = nc.dram_tensor("x", [NPC, C, H, W], BF16, kind="ExternalInput").ap()
    wt_d = nc.dram_tensor("wt", [9, C, K], FP8, kind="ExternalInput").ap()
    b_d = nc.dram_tensor("b2", [C, K // C], F32, kind="ExternalInput").ap()
    out_d = nc.dram_tensor("out", [NPC, K, H, W], F32, kind="ExternalOutput").ap()

    with tile.TileContext(nc) as tc:
        with (
            tc.tile_pool(name="const", bufs=1) as const_p,
            tc.tile_pool(name="raw", bufs=raw_bufs) as raw_p,
            tc.tile_pool(name="img", bufs=img_bufs) as img_p,
            tc.tile_pool(name="os", bufs=os_bufs) as os_p,
            tc.tile_pool(name="ps", bufs=psum_bufs, space="PSUM") as ps_p,
        ):
            wt = const_p.tile([C, 9, K], FP8, tag="wt")
            nc.sync.dma_start(out=wt[:], in_=wt_d[:].transpose([1, 0, 2]))
            bias = const_p.tile([C, K // C], F32, tag="bias")
            nc.sync.dma_start(out=bias[:], in_=b_d[:])

            if loop_n is not None:
                loop_ctx = tc.For_i(0, loop_n, 1,
                                    hint_engines=tuple(mybir.EngineType))
            else:
                loop_ctx = contextlib.nullcontext()
            with loop_ctx:
                body(nc, tc, bass, mybir, wt, bias, x_d, out_d, scale,
                     raw_p, img_p, os_p, ps_p, sign_split, evict,
                     out_dma, in_path)
    nc.compile()
    return nc


def body(nc, tc, bass, mybir, wt, bias, x_d, out_d, scale,
         raw_p, img_p, os_p, ps_p, sign_split=1, evict="alt",
         out_dma=True, in_path=True):
    FP8 = mybir.dt.float8e4
    F32 = mybir.dt.float32
    BF16 = mybir.dt.bfloat16
    DR = mybir.MatmulPerfMode.DoubleRow
    ACT_ID = mybir.ActivationFunctionType.Identity
    ACT_SIGN = mybir.ActivationFunctionType.Sign
    C = 128
    if True:
            for i in range(NPC):
                # --- build padded sign image ---
                if in_path:
                    raw = raw_p.tile([C, H, W], BF16, tag="raw")
                    nc.sync.dma_start(out=raw[:], in_=x_d[i])
                img = img_p.tile([C, IMG_LEN], FP8, tag="img")
                iap = img[:]
                pdim = list(iap.ap[0])  # [partition_stride, 128]

                def iview(off, ap_dims):
                    return bass.AP(tensor=iap.tensor, offset=iap.offset + off,
                                   ap=[pdim] + ap_dims)

                # zero the pad ring (and trailing garbage-read region)
                nc.vector.memset(iview(ALPHA, [[1, RS + 1]]), 0.0)  # row 0 + (1,0)
                nc.vector.memset(iview(ALPHA + 2 * RS - 1, [[RS, 55], [1, 2]]), 0.0)
                nc.vector.memset(iview(ALPHA + PLANE - RS - 1,
                                       [[1, IMG_LEN - ALPHA - PLANE + RS + 1]]), 0.0)
                # interior = sign(x), written strided into the padded buffer
                # (split into chunks so the first matmuls can start earlier)
                if in_path:
                    hc = H // sign_split
                    for s in range(sign_split):
                        nc.scalar.activation(
                            iview(ALPHA + RS * (1 + s * hc) + 1, [[RS, hc], [1, W]]),
                            raw[:, s * hc:(s + 1) * hc, :], ACT_SIGN)

                # --- conv: 2 k-tiles x 7 row-aligned spatial tiles ---
                for kt in range(K // C):
                    os = os_p.tile([C, H * W], F32, tag="os")
                    for t in range(NTILE):
                        s0 = ALPHA + RS + TS * t   # padded row 1+8t, col 0
                        ps = ps_p.tile([C, TS], F32, tag="ps")
                        for p in range(4):
                            a, b = OFF[2 * p], OFF[2 * p + 1]
                            rhs = iview(s0 + a, [[b - a, 2], [1, TS]])
                            lhsT = wt[:, 2 * p:2 * p + 2, kt * C:(kt + 1) * C]
                            nc.tensor.matmul(ps[:], lhsT, rhs, start=(p == 0),
                                             stop=False, perf_mode=DR)
                        rhs1 = iview(s0 + OFF[8], [[1, TS]])
                        nc.tensor.matmul(ps[:], wt[:, 8, kt * C:(kt + 1) * C],
                                         rhs1, start=False, stop=True)
                        # out = psum * (mean|x|*mean|w|) + bias[k], dropping the
                        # two pad columns; alternate ScalarE/VectorE for balance
                        src = ps[:].rearrange("p (r c) -> p r c", c=RS)[:, :, 1:W + 1]
                        dst = os[:, W * ROWS_PT * t:W * ROWS_PT * (t + 1)].rearrange(
                            "p (r c) -> p r c", c=W)
                        on_act = (t % 2 == 0) if evict == "alt" else (evict == "act")
                        if on_act:
                            nc.scalar.activation(dst, src, ACT_ID,
                                                 scale=float(scale),
                                                 bias=bias[:, kt:kt + 1])
                        else:
                            nc.vector.tensor_scalar(
                                dst, src, float(scale), bias[:, kt:kt + 1],
                                mybir.AluOpType.mult, mybir.AluOpType.add)
                    if out_dma or (i == NPC - 1 and kt == 1):
                        nc.sync.dma_start(
                            out=out_d[i, kt * C:(kt + 1) * C].rearrange(
                                "k h w -> k (h w)"),
                            in_=os[:])


def kernel(input: np.ndarray, weight: np.ndarray, bias: np.ndarray) -> np.ndarray:
    from concourse.bass_utils import run_bass_kernel_spmd

    x = np.ascontiguousarray(input, dtype=np.float32)
    w = np.asarray(weight, dtype=np.float32)
    b = np.asarray(bias, dtype=np.float32)

    # global binarization scalars (tiny, replicated); computed with CPU jax
    # to reproduce the reference's f32 jnp.mean reduction bit-for-bit
    import jax
    import jax.numpy as jnp
    with jax.default_device(jax.devices("cpu")[0]):
        sx = float(jnp.mean(jnp.abs(jnp.asarray(x))))
        sw = float(jnp.mean(jnp.abs(jnp.asarray(w))))
    scale = np.float32(sx) * np.float32(sw)

    # weights: sign, tap-major, transposed to [tap, C, K], fp8
    ws = np.sign(w)  # (K, C, 3, 3)
    wt = np.stack([ws[:, :, dy + 1, dx + 1].T for (dy, dx) in ORD])  # (9, C, K)
    wt = np.ascontiguousarray(wt).astype(ml_dtypes.float8_e4m3fn)
    b2 = np.ascontiguousarray(b.reshape(K // C, C).T)  # [C, 2]

    # ship input as bf16: sign() is invariant under bf16 rounding and the
    # device only consumes sign(x); halves the input DMA traffic
    xb = x.astype(ml_dtypes.bfloat16)

    nc = build_program(scale)
    in_maps = [
        {"x": xb[i * NPC:(i + 1) * NPC], "wt": wt, "b2": b2} for i in range(NCORES)
    ]
    res = run_bass_kernel_spmd(nc, in_maps, list(range(NCORES)))
    out = np.concatenate([res.results[i]["out"] for i in range(NCORES)], axis=0)
    return out.astype(np.float32)


if __name__ == "__main__":
    rng = np.random.default_rng(0)
    x = rng.normal(size=(N, C, H, W)).astype(np.float32)
    w = rng.normal(size=(K, C, KS, KS)).astype(np.float32)
    b = rng.normal(size=(K,)).astype(np.float32)
    o = kernel(input=x, weight=w, bias=b)
    print(o.shape, o.dtype)



# revision 2
# speedup vs baseline: 1.6213x; 1.6213x over previous
"""XNOR-Net style binarized 3x3 conv (BinConv2d) on 8 Trainium2 NeuronCores.

Math: out = conv(sign(x)*mean|x|, sign(w)*mean|w|) + b
         = (mean|x| * mean|w|) * conv(sign(x), sign(w)) + b

The conv operands are pure {-1, 0, +1}, exactly representable in fp8e4m3,
and all partial sums are small integers held exactly in fp32 PSUM, so the
heavy conv runs on the tensor engine in fp8 with DoubleRow (2x) perf mode.

Layout (per core, data-parallel over batch: 4 images/core):
  - input ships as bf16 (sign() is invariant under bf16 rounding); sign is
    computed on ScalarE into a padded fp8 image: 58x58 rows/cols (pad=1
    ring of zeros), flattened per channel partition.
  - conv = 9 shifted matmuls over the flat image, C=128 as the partition/
    contraction dim; the 9 taps run as 4 DoubleRow pair-matmuls + 1 plain.
  - spatial tiling: 8 tiles of 7 output rows; the matmul moving AP skips
    the pad columns ([[pair],[58,7],[1,56]]), so PSUM subtiles are dense
    392 px and only real output pixels are computed.
  - 2 subtiles share one [C, 1024] PSUM tile (2 banks); ONE tensor_scalar
    instruction drains both banks fused with scale (mean|x|*mean|w|) and
    bias (the per-instruction fixed cost of PSUM evictions, ~0.45us, is
    the 2nd-biggest engine load after the PE).
  - evictions run on VectorE; ScalarE only does sign, and the input stage
    for image i+1 is issued ahead of image i's conv so the in-order
    ScalarE queue never delays the PE at image boundaries.
  - output staged and DMA'd as bf16 (halves output traffic; exact conv
    ints are scaled in f32 and only rounded on the final store; host casts
    back to f32). Relative error vs the f32 reference ~1.6e-3.

Measured ~65-70us/core steady-state on HW (PE-bound: 320 matmuls/body at
the fp8 DoubleRow rate ~ 58us, input/output DMA, sign and evictions
overlapped).
"""

import numpy as np
import ml_dtypes

# Problem constants (hardcoded per contest rules)
N, C, H, W = 32, 128, 56, 56
K, KS = 256, 3
NCORES = 8
NPC = N // NCORES          # images per core
RS = H + 2                 # padded row stride = 58
PLANE = RS * RS            # 3364
IMG_LEN = 3712             # padded plane + guard slack
ALPHA = 16                 # image plane base offset inside the SBUF buffer
TS = 7 * RS                # spatial tile = 7 padded rows = 406
NTILE = 8                  # 8 tiles x 7 rows cover output rows 0..55
ROWS_PT = 7
BANK = 512                 # f32 elements per PSUM bank

# tap order: pairs (0,1),(2,3),(4,5),(6,7) are DoubleRow pairs; 8 is single
ORD = [(-1, -1), (-1, 0), (-1, 1), (0, -1), (0, 0), (0, 1), (1, -1), (1, 0), (1, 1)]
OFF = [dy * RS + dx for (dy, dx) in ORD]


def build_program(scale: float, loop_n: int | None = None, grp: int = 2,
                  os_bufs: int = 4, sign_split: int = 2,
                  ev_eng: str = "dve", out_dma: bool = True,
                  img_bufs: int = 4, raw_bufs: int = 2, obf16: bool = True,
                  staggered: bool = True, prefetch: bool = True,
                  unroll: int = 1):
    """Build the per-core program. loop_n: if set, wrap the body in a
    hardware For loop (timing-harness variant; `unroll` bodies per
    iteration, loop_n total bodies)."""
    if loop_n is not None:
        assert loop_n % unroll == 0
        loop_n = loop_n // unroll
    import contextlib
    from concourse import bass, bacc, tile, mybir

    FP8 = mybir.dt.float8e4
    F32 = mybir.dt.float32
    BF16 = mybir.dt.bfloat16
    ACT_SIGN = mybir.ActivationFunctionType.Sign
    psum_bufs = 8 // grp

    nc = bacc.Bacc("TRN2", target_bir_lowering=False, debug=False)
    x_d = nc.dram_tensor("x", [NPC, C, H, W], BF16, kind="ExternalInput").ap()
    wt_d = nc.dram_tensor("wt", [9, C, K], FP8, kind="ExternalInput").ap()
    b_d = nc.dram_tensor("b2", [C, K // C], F32, kind="ExternalInput").ap()
    out_dt = BF16 if obf16 else F32
    out_d = nc.dram_tensor("out", [NPC, K, H, W], out_dt,
                           kind="ExternalOutput").ap()

    with tile.TileContext(nc) as tc:
        with (
            tc.tile_pool(name="const", bufs=1) as const_p,
            tc.tile_pool(name="raw", bufs=raw_bufs) as raw_p,
            tc.tile_pool(name="img", bufs=img_bufs) as img_p,
            tc.tile_pool(name="os", bufs=os_bufs) as os_p,
            tc.tile_pool(name="ps", bufs=psum_bufs, space="PSUM") as ps_p,
        ):
            wt = const_p.tile([C, 9, K], FP8, tag="wt")
            nc.sync.dma_start(out=wt[:], in_=wt_d[:].transpose([1, 0, 2]))
            bias = const_p.tile([C, K // C], F32, tag="bias")
            nc.sync.dma_start(out=bias[:], in_=b_d[:])

            ctr = [0]

            def input_stage(i):
                """DMA raw image i, zero pad ring, sign into an img tile."""
                n = ctr[0]
                ctr[0] += 1
                raw = raw_p.tile([C, H, W], BF16, tag="raw", name=f"raw{n}")
                nc.sync.dma_start(out=raw[:], in_=x_d[i])
                img = img_p.tile([C, IMG_LEN], FP8, tag="img", name=f"img{n}")
                iap = img[:]
                pdim = list(iap.ap[0])

                def iview(off, ap_dims):
                    return bass.AP(tensor=iap.tensor, offset=iap.offset + off,
                                   ap=[pdim] + ap_dims)

                nc.gpsimd.memset(iview(ALPHA, [[1, RS + 1]]), 0.0)
                nc.gpsimd.memset(iview(ALPHA + 2 * RS - 1, [[RS, 55], [1, 2]]),
                                 0.0)
                nc.gpsimd.memset(iview(ALPHA + PLANE - RS - 1,
                                       [[1, IMG_LEN - ALPHA - PLANE + RS + 1]]),
                                 0.0)
                hc = H // sign_split
                for s in range(sign_split):
                    nc.scalar.activation(
                        iview(ALPHA + RS * (1 + s * hc) + 1,
                              [[RS, hc], [1, W]]),
                        raw[:, s * hc:(s + 1) * hc, :], ACT_SIGN)
                return iview

            iv0 = input_stage(0) if prefetch else None

            if loop_n is not None:
                loop_ctx = tc.For_i(0, loop_n, 1,
                                    hint_engines=tuple(mybir.EngineType),
                                    staggered_reset=staggered)
            else:
                loop_ctx = contextlib.nullcontext()
            with loop_ctx:
                for _u in range(unroll if loop_n is not None else 1):
                    body(nc, tc, bass, mybir, wt, bias, x_d, out_d, scale,
                         os_p, ps_p, input_stage, iv0, ev_eng, out_dma,
                         out_dt, grp)
    nc.compile()
    return nc


def body(nc, tc, bass, mybir, wt, bias, x_d, out_d, scale,
         os_p, ps_p, input_stage, iv0, ev_eng, out_dma, out_dt, GRP):
    NGRP = NTILE // GRP
    F32 = mybir.dt.float32
    DR = mybir.MatmulPerfMode.DoubleRow
    ACT_ID = mybir.ActivationFunctionType.Identity
    C = 128
    OS_LEN = H * W
    ev_count = 0

    iviews = {0: iv0 if iv0 is not None else input_stage(0)}
    for i in range(NPC):
        if i + 1 < NPC:
            iviews[i + 1] = input_stage(i + 1)
        elif iv0 is not None:
            # prefetch image 0 for the next loop iteration / body
            input_stage(0)
        iview = iviews.pop(i)
        for kt in range(K // C):
            os = os_p.tile([C, OS_LEN], out_dt, tag="os")
            for g in range(NGRP):
                ps = ps_p.tile([C, GRP * BANK], F32, tag="ps")
                pap = ps[:]
                ppd = list(pap.ap[0])
                # 4 DoubleRow pair-sweeps over the GRP subtiles
                for p in range(4):
                    a, b = OFF[2 * p], OFF[2 * p + 1]
                    lhsT = wt[:, 2 * p:2 * p + 2, kt * C:(kt + 1) * C]
                    for t in range(GRP):
                        s0 = ALPHA + RS + TS * (GRP * g + t) + 1
                        rhs = iview(s0 + a, [[b - a, 2], [RS, ROWS_PT], [1, W]])
                        out_ap = bass.AP(tensor=pap.tensor,
                                         offset=pap.offset + BANK * t,
                                         ap=[ppd, [1, W * ROWS_PT]])
                        nc.tensor.matmul(out_ap, lhsT, rhs, start=(p == 0),
                                         stop=False, perf_mode=DR)
                # 9th tap, then one fused scale+bias drain of all GRP banks
                lhsT8 = wt[:, 8, kt * C:(kt + 1) * C]
                for t in range(GRP):
                    s0 = ALPHA + RS + TS * (GRP * g + t) + 1
                    rhs1 = iview(s0 + OFF[8], [[RS, ROWS_PT], [1, W]])
                    out_ap = bass.AP(tensor=pap.tensor,
                                     offset=pap.offset + BANK * t,
                                     ap=[ppd, [1, W * ROWS_PT]])
                    nc.tensor.matmul(out_ap, lhsT8, rhs1, start=False,
                                     stop=True)
                oap = os[:]
                opd = list(oap.ap[0])
                src = bass.AP(tensor=pap.tensor, offset=pap.offset,
                              ap=[ppd, [BANK, GRP], [1, W * ROWS_PT]])
                dst = bass.AP(tensor=oap.tensor,
                              offset=oap.offset + W * ROWS_PT * GRP * g,
                              ap=[opd, [W * ROWS_PT, GRP], [1, W * ROWS_PT]])
                on_act = (ev_count % 2 == 0) if ev_eng == "alt" else False
                ev_count += 1
                if on_act:
                    nc.scalar.activation(dst, src, ACT_ID, scale=float(scale),
                                         bias=bias[:, kt:kt + 1])
                else:
                    nc.vector.tensor_scalar(dst, src, float(scale),
                                            bias[:, kt:kt + 1],
                                            mybir.AluOpType.mult,
                                            mybir.AluOpType.add)
            if out_dma or (i == NPC - 1 and kt == 1):
                nc.sync.dma_start(
                    out=out_d[i, kt * C:(kt + 1) * C].rearrange(
                        "k h w -> k (h w)"),
                    in_=os[:])


def kernel(input: np.ndarray, weight: np.ndarray, bias: np.ndarray) -> np.ndarray:
    from concourse.bass_utils import run_bass_kernel_spmd

    x = np.ascontiguousarray(input, dtype=np.float32)
    w = np.asarray(weight, dtype=np.float32)
    b = np.asarray(bias, dtype=np.float32)

    # global binarization scalars (tiny, replicated); computed with CPU jax
    # to reproduce the reference's f32 jnp.mean reduction bit-for-bit
    import jax
    import jax.numpy as jnp
    with jax.default_device(jax.devices("cpu")[0]):
        sx = float(jnp.mean(jnp.abs(jnp.asarray(x))))
        sw = float(jnp.mean(jnp.abs(jnp.asarray(w))))
    scale = np.float32(sx) * np.float32(sw)

    # weights: sign, tap-major, transposed to [tap, C, K], fp8
    ws = np.sign(w)  # (K, C, 3, 3)
    wt = np.stack([ws[:, :, dy + 1, dx + 1].T for (dy, dx) in ORD])  # (9, C, K)
    wt = np.ascontiguousarray(wt).astype(ml_dtypes.float8_e4m3fn)
    b2 = np.ascontiguousarray(b.reshape(K // C, C).T)  # [C, 2]

    # ship input as bf16: sign() is invariant under bf16 rounding and the
    # device only consumes sign(x); halves the input DMA traffic
    xb = x.astype(ml_dtypes.bfloat16)

    nc = build_program(scale)
    in_maps = [
        {"x": xb[i * NPC:(i + 1) * NPC], "wt": wt, "b2": b2} for i in range(NCORES)
    ]
    res = run_bass_kernel_spmd(nc, in_maps, list(range(NCORES)))
    out = np.concatenate([res.results[i]["out"] for i in range(NCORES)], axis=0)
    return out.astype(np.float32)


if __name__ == "__main__":
    rng = np.random.default_rng(0)
    x = rng.normal(size=(N, C, H, W)).astype(np.float32)
    w = rng.normal(size=(K, C, KS, KS)).astype(np.float32)
    b = rng.normal(size=(K,)).astype(np.float32)
    o = kernel(input=x, weight=w, bias=b)
    print(o.shape, o.dtype)


# revision 4
# speedup vs baseline: 1.6519x; 1.0189x over previous
"""XNOR-Net style binarized 3x3 conv (BinConv2d) on 8 Trainium2 NeuronCores.

Math: out = conv(sign(x)*mean|x|, sign(w)*mean|w|) + b
         = (mean|x| * mean|w|) * conv(sign(x), sign(w)) + b

The conv operands are pure {-1, 0, +1}, exactly representable in fp8e4m3,
and all partial sums are small integers held exactly in fp32 PSUM, so the
heavy conv runs on the tensor engine in fp8 with DoubleRow (2x) perf mode.

Layout (per core, data-parallel over batch: 4 images/core):
  - input ships as bf16 (sign() is invariant under bf16 rounding); sign is
    computed on ScalarE into a padded fp8 image: 58x58 rows/cols (pad=1
    ring of zeros), flattened per channel partition.
  - conv = 9 shifted matmuls over the flat image, C=128 as the partition/
    contraction dim; the 9 taps run as 4 DoubleRow pair-matmuls + 1 plain.
  - spatial tiling: 8 tiles of 7 output rows; the matmul moving AP skips
    the pad columns ([[pair],[58,7],[1,56]]), so PSUM subtiles are dense
    392 px and only real output pixels are computed.
  - 2 subtiles share one [C, 1024] PSUM tile (2 banks); ONE tensor_scalar
    instruction drains both banks fused with scale (mean|x|*mean|w|) and
    bias (the per-instruction fixed cost of PSUM evictions, ~0.45us, is
    the 2nd-biggest engine load after the PE).
  - evictions run on VectorE; ScalarE only does sign, and the input stage
    for image i+1 is issued ahead of image i's conv so the in-order
    ScalarE queue never delays the PE at image boundaries.
  - output staged and DMA'd as bf16 (halves output traffic; exact conv
    ints are scaled in f32 and only rounded on the final store; host casts
    back to f32). Relative error vs the f32 reference ~1.6e-3.

Measured ~55-56us/core steady-state on HW — equal to a matmul-only
program with the same instruction stream (PE-bound at the fp8 DoubleRow
rate; input/output DMA, sign and evictions fully overlapped).
"""

import numpy as np
import ml_dtypes

# Problem constants (hardcoded per contest rules)
N, C, H, W = 32, 128, 56, 56
K, KS = 256, 3
NCORES = 8
NPC = N // NCORES          # images per core
RS = H + 2                 # padded row stride = 58
PLANE = RS * RS            # 3364
IMG_LEN = 3712             # padded plane + guard slack
ALPHA = 16                 # image plane base offset inside the SBUF buffer
TS = 7 * RS                # spatial tile = 7 padded rows = 406
NTILE = 8                  # 8 tiles x 7 rows cover output rows 0..55
ROWS_PT = 7
BANK = 512                 # f32 elements per PSUM bank

# tap order: pairs (0,1),(2,3),(4,5),(6,7) are DoubleRow pairs; 8 is single
ORD = [(-1, -1), (-1, 0), (-1, 1), (0, -1), (0, 0), (0, 1), (1, -1), (1, 0), (1, 1)]
OFF = [dy * RS + dx for (dy, dx) in ORD]


def build_program(scale: float, loop_n: int | None = None, grp: int = 2,
                  os_bufs: int = 4, sign_split: int = 1,
                  ev_eng: str = "dve", out_dma: bool = True,
                  img_bufs: int = 4, raw_bufs: int = 2, obf16: bool = True,
                  staggered: bool = True, prefetch: bool = True,
                  unroll: int = 1):
    """Build the per-core program. loop_n: if set, wrap the body in a
    hardware For loop (timing-harness variant; `unroll` bodies per
    iteration, loop_n total bodies)."""
    if loop_n is not None:
        assert loop_n % unroll == 0
        loop_n = loop_n // unroll
    import contextlib
    from concourse import bass, bacc, tile, mybir

    FP8 = mybir.dt.float8e4
    F32 = mybir.dt.float32
    BF16 = mybir.dt.bfloat16
    ACT_SIGN = mybir.ActivationFunctionType.Sign
    psum_bufs = 8 // grp

    nc = bacc.Bacc("TRN2", target_bir_lowering=False, debug=False)
    x_d = nc.dram_tensor("x", [NPC, C, H, W], BF16, kind="ExternalInput").ap()
    wt_d = nc.dram_tensor("wt", [9, C, K], FP8, kind="ExternalInput").ap()
    b_d = nc.dram_tensor("b2", [C, K // C], F32, kind="ExternalInput").ap()
    out_dt = BF16 if obf16 else F32
    out_d = nc.dram_tensor("out", [NPC, K, H, W], out_dt,
                           kind="ExternalOutput").ap()

    with tile.TileContext(nc) as tc:
        with (
            tc.tile_pool(name="const", bufs=1) as const_p,
            tc.tile_pool(name="raw", bufs=raw_bufs) as raw_p,
            tc.tile_pool(name="img", bufs=img_bufs) as img_p,
            tc.tile_pool(name="os", bufs=os_bufs) as os_p,
            tc.tile_pool(name="ps", bufs=psum_bufs, space="PSUM") as ps_p,
        ):
            wt = const_p.tile([C, 9, K], FP8, tag="wt")
            nc.sync.dma_start(out=wt[:], in_=wt_d[:].transpose([1, 0, 2]))
            bias = const_p.tile([C, K // C], F32, tag="bias")
            nc.sync.dma_start(out=bias[:], in_=b_d[:])

            ctr = [0]

            def input_stage(i):
                """DMA raw image i, zero pad ring, sign into an img tile."""
                n = ctr[0]
                ctr[0] += 1
                raw = raw_p.tile([C, H, W], BF16, tag="raw", name=f"raw{n}")
                nc.sync.dma_start(out=raw[:], in_=x_d[i])
                img = img_p.tile([C, IMG_LEN], FP8, tag="img", name=f"img{n}")
                iap = img[:]
                pdim = list(iap.ap[0])

                def iview(off, ap_dims):
                    return bass.AP(tensor=iap.tensor, offset=iap.offset + off,
                                   ap=[pdim] + ap_dims)

                nc.gpsimd.memset(iview(ALPHA, [[1, RS + 1]]), 0.0)
                nc.gpsimd.memset(iview(ALPHA + 2 * RS - 1, [[RS, 55], [1, 2]]),
                                 0.0)
                nc.gpsimd.memset(iview(ALPHA + PLANE - RS - 1,
                                       [[1, IMG_LEN - ALPHA - PLANE + RS + 1]]),
                                 0.0)
                hc = H // sign_split
                for s in range(sign_split):
                    nc.scalar.activation(
                        iview(ALPHA + RS * (1 + s * hc) + 1,
                              [[RS, hc], [1, W]]),
                        raw[:, s * hc:(s + 1) * hc, :], ACT_SIGN)
                return iview

            iv0 = input_stage(0) if prefetch else None

            if loop_n is not None:
                loop_ctx = tc.For_i(0, loop_n, 1,
                                    hint_engines=tuple(mybir.EngineType),
                                    staggered_reset=staggered)
            else:
                loop_ctx = contextlib.nullcontext()
            with loop_ctx:
                for _u in range(unroll if loop_n is not None else 1):
                    body(nc, tc, bass, mybir, wt, bias, x_d, out_d, scale,
                         os_p, ps_p, input_stage, iv0, ev_eng, out_dma,
                         out_dt, grp)
    nc.compile()
    return nc


def body(nc, tc, bass, mybir, wt, bias, x_d, out_d, scale,
         os_p, ps_p, input_stage, iv0, ev_eng, out_dma, out_dt, GRP):
    NGRP = NTILE // GRP
    F32 = mybir.dt.float32
    DR = mybir.MatmulPerfMode.DoubleRow
    ACT_ID = mybir.ActivationFunctionType.Identity
    C = 128
    OS_LEN = H * W
    ev_count = 0

    iviews = {0: iv0 if iv0 is not None else input_stage(0)}
    for i in range(NPC):
        if i + 1 < NPC:
            iviews[i + 1] = input_stage(i + 1)
        elif iv0 is not None:
            # prefetch image 0 for the next loop iteration / body
            input_stage(0)
        iview = iviews.pop(i)
        for kt in range(K // C):
            os = os_p.tile([C, OS_LEN], out_dt, tag="os")
            for g in range(NGRP):
                ps = ps_p.tile([C, GRP * BANK], F32, tag="ps")
                pap = ps[:]
                ppd = list(pap.ap[0])
                # 4 DoubleRow pair-sweeps over the GRP subtiles
                for p in range(4):
                    a, b = OFF[2 * p], OFF[2 * p + 1]
                    lhsT = wt[:, 2 * p:2 * p + 2, kt * C:(kt + 1) * C]
                    for t in range(GRP):
                        s0 = ALPHA + RS + TS * (GRP * g + t) + 1
                        rhs = iview(s0 + a, [[b - a, 2], [RS, ROWS_PT], [1, W]])
                        out_ap = bass.AP(tensor=pap.tensor,
                                         offset=pap.offset + BANK * t,
                                         ap=[ppd, [1, W * ROWS_PT]])
                        nc.tensor.matmul(out_ap, lhsT, rhs, start=(p == 0),
                                         stop=False, perf_mode=DR)
                # 9th tap, then one fused scale+bias drain of all GRP banks
                lhsT8 = wt[:, 8, kt * C:(kt + 1) * C]
                for t in range(GRP):
                    s0 = ALPHA + RS + TS * (GRP * g + t) + 1
                    rhs1 = iview(s0 + OFF[8], [[RS, ROWS_PT], [1, W]])
                    out_ap = bass.AP(tensor=pap.tensor,
                                     offset=pap.offset + BANK * t,
                                     ap=[ppd, [1, W * ROWS_PT]])
                    nc.tensor.matmul(out_ap, lhsT8, rhs1, start=False,
                                     stop=True)
                oap = os[:]
                opd = list(oap.ap[0])
                src = bass.AP(tensor=pap.tensor, offset=pap.offset,
                              ap=[ppd, [BANK, GRP], [1, W * ROWS_PT]])
                dst = bass.AP(tensor=oap.tensor,
                              offset=oap.offset + W * ROWS_PT * GRP * g,
                              ap=[opd, [W * ROWS_PT, GRP], [1, W * ROWS_PT]])
                on_act = (ev_count % 2 == 0) if ev_eng == "alt" else False
                ev_count += 1
                if on_act:
                    nc.scalar.activation(dst, src, ACT_ID, scale=float(scale),
                                         bias=bias[:, kt:kt + 1])
                else:
                    nc.vector.tensor_scalar(dst, src, float(scale),
                                            bias[:, kt:kt + 1],
                                            mybir.AluOpType.mult,
                                            mybir.AluOpType.add)
            if out_dma or (i == NPC - 1 and kt == 1):
                nc.sync.dma_start(
                    out=out_d[i, kt * C:(kt + 1) * C].rearrange(
                        "k h w -> k (h w)"),
                    in_=os[:])


def kernel(input: np.ndarray, weight: np.ndarray, bias: np.ndarray) -> np.ndarray:
    from concourse.bass_utils import run_bass_kernel_spmd

    x = np.ascontiguousarray(input, dtype=np.float32)
    w = np.asarray(weight, dtype=np.float32)
    b = np.asarray(bias, dtype=np.float32)

    # global binarization scalars (tiny, replicated); computed with CPU jax
    # to reproduce the reference's f32 jnp.mean reduction bit-for-bit
    import jax
    import jax.numpy as jnp
    with jax.default_device(jax.devices("cpu")[0]):
        sx = float(jnp.mean(jnp.abs(jnp.asarray(x))))
        sw = float(jnp.mean(jnp.abs(jnp.asarray(w))))
    scale = np.float32(sx) * np.float32(sw)

    # weights: sign, tap-major, transposed to [tap, C, K], fp8
    ws = np.sign(w)  # (K, C, 3, 3)
    wt = np.stack([ws[:, :, dy + 1, dx + 1].T for (dy, dx) in ORD])  # (9, C, K)
    wt = np.ascontiguousarray(wt).astype(ml_dtypes.float8_e4m3fn)
    b2 = np.ascontiguousarray(b.reshape(K // C, C).T)  # [C, 2]

    # ship input as bf16: sign() is invariant under bf16 rounding and the
    # device only consumes sign(x); halves the input DMA traffic
    xb = x.astype(ml_dtypes.bfloat16)

    nc = build_program(scale)
    in_maps = [
        {"x": xb[i * NPC:(i + 1) * NPC], "wt": wt, "b2": b2} for i in range(NCORES)
    ]
    res = run_bass_kernel_spmd(nc, in_maps, list(range(NCORES)))
    out = np.concatenate([res.results[i]["out"] for i in range(NCORES)], axis=0)
    return out.astype(np.float32)


if __name__ == "__main__":
    rng = np.random.default_rng(0)
    x = rng.normal(size=(N, C, H, W)).astype(np.float32)
    w = rng.normal(size=(K, C, KS, KS)).astype(np.float32)
    b = rng.normal(size=(K,)).astype(np.float32)
    o = kernel(input=x, weight=w, bias=b)
    print(o.shape, o.dtype)
